# revision 1
# baseline (speedup 1.0000x reference)
"""Trainium2 Bass kernel for batched single-query attention over ragged
sequences.

Problem: query (N,D), key (N,T,D), value (N,T,V), lens (N,) with
N=64, T=2048, D=V=256.  Returns (context (N,V), attention (N,T)).

Strategy: data-parallel over N across 8 NeuronCores (8 rows per core).
Host-side we transpose key to (N, D, T) so the energy matvec can run on
the TensorEngine with d on partitions, pack query/mask into
SBUF-resident layouts, and bin-pack rows to cores by ceil(lens/128) so
every core does a similar amount of work.  The program is specialized
(and cached) per per-slot chunk-count profile so only the valid prefix
of each row's key/value is ever read from HBM.
"""

import numpy as np

N_CORES = 8
N, T, D, V = 64, 2048, 256, 256
PT = 128                 # partition count / t-chunk size
TC = T // PT             # 16 chunks max per row
SLOTS = N // N_CORES     # 8 rows per core
NEG_INF = -1e9

_program_cache: dict = {}


def _build(k_slots, reps=1):
    """Build + compile the SPMD Bass program.

    k_slots: per-slot chunk counts (len SLOTS); slot i on every core
    processes the first k_slots[i]*128 positions of its row.
    reps: unroll the whole per-core computation this many times
    (identical work; used for on-HW timing by differencing).
    """
    import concourse.tile as tile
    from concourse import bacc, mybir
    from concourse.masks import make_identity

    f32 = mybir.dt.float32
    AX = mybir.AxisListType
    ACT = mybir.ActivationFunctionType

    nc = bacc.Bacc(
        "TRN2", target_bir_lowering=False, debug=False, num_devices=N_CORES
    )

    keyT_d = nc.dram_tensor("keyT", (SLOTS, D, T), f32, kind="ExternalInput")
    val_d = nc.dram_tensor("val", (SLOTS, T, V), f32, kind="ExternalInput")
    q_d = nc.dram_tensor("qpk", (PT, SLOTS, 2), f32, kind="ExternalInput")
    m_d = nc.dram_tensor("maskpk", (PT, SLOTS, TC), f32, kind="ExternalInput")
    ctx_d = nc.dram_tensor("ctx", (SLOTS, V), f32, kind="ExternalOutput")
    att_d = nc.dram_tensor("att", (SLOTS, T), f32, kind="ExternalOutput")

    keyT_ap = keyT_d.ap().rearrange("s (dc p) t -> s p dc t", p=PT)
    val_ap = val_d.ap().rearrange("s (c p) v -> s p c v", p=PT)
    att_ap = att_d.ap().rearrange("s (c p) -> s c p", p=PT)

    with tile.TileContext(nc) as tc:
        with (
            tc.tile_pool(name="const", bufs=1) as constp,
            tc.tile_pool(name="kp", bufs=2) as kp,
            tc.tile_pool(name="vp", bufs=2) as vp,
            tc.tile_pool(name="sp", bufs=3) as sp,
            tc.tile_pool(name="pe", bufs=2, space="PSUM") as pe_pool,
            tc.tile_pool(name="pm", bufs=2, space="PSUM") as pm_pool,
            tc.tile_pool(name="pc", bufs=2, space="PSUM") as pc_pool,
        ):
            ident = constp.tile([PT, PT], f32)
            make_identity(nc, ident[:])
            ones = constp.tile([PT, 1], f32)
            nc.gpsimd.memset(ones[:], 1.0)
            qsb = constp.tile([PT, SLOTS, 2], f32)
            nc.sync.dma_start(qsb[:], q_d.ap())
            msb = constp.tile([PT, SLOTS, TC], f32)
            nc.sync.dma_start(msb[:], m_d.ap())

            for _ in range(reps):
                for i in range(SLOTS):
                    K = k_slots[i]
                    kt = kp.tile([PT, 2, PT * K], f32, tag="kt")
                    nc.sync.dma_start(kt[:], keyT_ap[i, :, :, 0 : PT * K])
                    vt = vp.tile([PT, K, V], f32, tag="vt")
                    nc.sync.dma_start(vt[:], val_ap[i, :, 0:K, :])

                    # energy[t] = sum_d keyT[d, t] * q[d], chunked
                    p_e = pe_pool.tile([PT, K], f32, tag="pe")
                    for c in range(K):
                        for dc in range(2):
                            nc.tensor.matmul(
                                p_e[:, c : c + 1],
                                lhsT=kt[:, dc, PT * c : PT * (c + 1)],
                                rhs=qsb[:, i, dc : dc + 1],
                                start=(dc == 0),
                                stop=(dc == 1),
                            )

                    # mask (additive 0 / -1e9)
                    e_sb = sp.tile([PT, K], f32, tag="e")
                    nc.vector.tensor_add(e_sb[:], p_e[:], msb[:, i, 0:K])

                    # global max over the [PT, K] tile
                    rmax = sp.tile([PT, 1], f32, tag="rmax")
                    nc.vector.reduce_max(rmax[:], e_sb[:], axis=AX.X)
                    p_t1 = pm_pool.tile([1, PT], f32, tag="pm")
                    nc.tensor.transpose(p_t1[:], rmax[:], ident[:])
                    nmax = sp.tile([1, 1], f32, tag="nmax")
                    nc.vector.reduce_max(nmax[:], p_t1[:], axis=AX.X, negate=True)
                    nmax_b = sp.tile([PT, 1], f32, tag="nmaxb")
                    nc.gpsimd.partition_broadcast(nmax_b[:], nmax[:])

                    # exp(e - max) with fused per-partition sum
                    attn = sp.tile([PT, K], f32, tag="attn")
                    rsum = sp.tile([PT, 1], f32, tag="rsum")
                    nc.scalar.activation(
                        attn[:], e_sb[:], ACT.Exp, bias=nmax_b[:], accum_out=rsum[:]
                    )
                    p_s = pm_pool.tile([1, 1], f32, tag="pm")
                    nc.tensor.matmul(
                        p_s[:], lhsT=rsum[:], rhs=ones[:], start=True, stop=True
                    )
                    rcp = sp.tile([1, 1], f32, tag="rcp")
                    nc.vector.reciprocal(rcp[:], p_s[:])

                    # context = sum_t attn[t] * value[t, :] (unnormalized,
                    # scaled by 1/sum at the end)
                    p_ctx = pc_pool.tile([1, V], f32, tag="pc")
                    for c in range(K):
                        nc.tensor.matmul(
                            p_ctx[:],
                            lhsT=attn[:, c : c + 1],
                            rhs=vt[:, c, :],
                            start=(c == 0),
                            stop=(c == K - 1),
                        )
                    ctx_sb = sp.tile([1, V], f32, tag="ctx")
                    nc.vector.tensor_scalar_mul(ctx_sb[:], p_ctx[:], rcp[:])
                    nc.sync.dma_start(ctx_d.ap()[i : i + 1, :], ctx_sb[:])

                    # normalized attention, transposed to [K, 128] rows for
                    # a contiguous DMA out
                    rcp_b = sp.tile([PT, 1], f32, tag="rcpb")
                    nc.gpsimd.partition_broadcast(rcp_b[:], rcp[:])
                    attn_n = sp.tile([PT, K], f32, tag="attnn")
                    nc.vector.tensor_scalar_mul(attn_n[:], attn[:], rcp_b[:])
                    p_at = pm_pool.tile([K, PT], f32, tag="pat")
                    nc.tensor.transpose(p_at[:], attn_n[:], ident[:])
                    attn_t = sp.tile([K, PT], f32, tag="attnt")
                    nc.vector.tensor_copy(attn_t[:], p_at[:])
                    nc.sync.dma_start(att_ap[i, 0:K, :], attn_t[:])

    nc.compile()
    return nc


def _get_program(k_slots, reps=1):
    key = (tuple(k_slots), reps)
    if key not in _program_cache:
        _program_cache[key] = _build(k_slots, reps)
    return _program_cache[key]


def _plan(lens):
    """Assign rows to (core, slot) balancing chunk counts.

    Sort rows by chunk count desc; slot i takes ranks [8i, 8i+8) spread
    across the 8 cores, so the per-slot max (which sets the compiled
    chunk count) is tight.
    Returns (assign[core][slot] -> n, k_slots[slot]).
    """
    cn = np.minimum((np.asarray(lens) + PT - 1) // PT, TC).astype(int)
    cn = np.maximum(cn, 1)
    order = np.argsort(-cn, kind="stable")
    assign = [[0] * SLOTS for _ in range(N_CORES)]
    k_slots = [0] * SLOTS
    for i in range(SLOTS):
        grp = order[i * N_CORES : (i + 1) * N_CORES]
        k_slots[i] = int(cn[grp].max())
        for c in range(N_CORES):
            assign[c][i] = int(grp[c])
    return assign, k_slots


def _pack_inputs(query, key, value, lens, assign):
    keyT = np.ascontiguousarray(np.transpose(key, (0, 2, 1)))
    t_idx = np.arange(T, dtype=np.int64)
    in_maps = []
    for c in range(N_CORES):
        ns = assign[c]
        qpk = np.ascontiguousarray(
            query[ns].reshape(SLOTS, 2, PT).transpose(2, 0, 1)
        )
        mask = np.where(
            t_idx[None, :] >= np.asarray(lens)[ns][:, None], np.float32(NEG_INF), 0.0
        ).astype(np.float32)
        mpk = np.ascontiguousarray(
            mask.reshape(SLOTS, TC, PT).transpose(2, 0, 1)
        )
        in_maps.append(
            {
                "keyT": np.ascontiguousarray(keyT[ns]),
                "val": np.ascontiguousarray(value[ns]),
                "qpk": qpk,
                "maskpk": mpk,
            }
        )
    return in_maps


def kernel(query, key, value, lens):
    from concourse import bass_utils

    query = np.asarray(query, dtype=np.float32)
    key = np.asarray(key, dtype=np.float32)
    value = np.asarray(value, dtype=np.float32)
    lens = np.asarray(lens)

    assign, k_slots = _plan(lens)
    nc = _get_program(k_slots)
    in_maps = _pack_inputs(query, key, value, lens, assign)
    res = bass_utils.run_bass_kernel_spmd(
        nc, in_maps, core_ids=list(range(N_CORES))
    )

    context = np.zeros((N, V), dtype=np.float32)
    attention = np.zeros((N, T), dtype=np.float32)
    for c in range(N_CORES):
        for i in range(SLOTS):
            n = assign[c][i]
            context[n] = res.results[c]["ctx"][i]
            attention[n] = res.results[c]["att"][i]
    return (context, attention)
